# revision 1
# baseline (speedup 1.0000x reference)
"""Bilateral-filter L1 loss kernel for 8 Trainium2 NeuronCores.

Math (per image tensor X, with a = 0.5 if X.min()<0 else 1.0, b = (1-a)... ):
  reference filters X01 = a*X + b (b=0.5a or 0) with a 5x5 bilateral kernel.
  range weight: exp(-((p01-c01)^2)/(2*0.1)) = exp(-gamma*(p-c)^2), gamma=a^2/0.2
  filtered01 = a*(c + U/W) + b  where, in RAW pixel domain,
     W = sum_k s_k * r_k,  U = sum_k s_k * r_k * (p_k - c)
  loss = mean |f_o - f_t| = a * mean |(c_o - c_t) + U_o/W_o - U_t/W_t|  (a_o==a_t)

Device pipeline per (channel, 128-row block) unit:
  u = p - c            (DVE fp16, all 25 taps, overlapped-window APs)
  d2 = u*u             (split ACT Square / DVE mult)
  r = exp(-gamma*d2)   (ACT, in-place over d2)
  m = r*u              (DVE, in-place over u)
  W += (s_k*I).T @ r_k ; U += (s_k*I).T @ m_k   (PE identity matmuls, fp32 PSUM)
  finalize: diff = (c_o-c_t) + U_o/W_o - U_t/W_t ; loss_col = sum |diff|
Host: shards inputs (24 (channel,rowblock) pairs -> 3 per core), sums partials.
"""

import numpy as np
from contextlib import ExitStack

B, C, H, W = 2, 3, 512, 512
KS, PAD = 5, 2
PW = W + 2 * PAD            # 516 padded width
NTAP = KS * KS
NCORES = 8
RB = H // 128               # 4 row blocks per channel
NCH = B * C                 # 6 channels per image tensor
PAIRS = NCH * RB            # 24
PPC = PAIRS // NCORES       # 3 pairs per core
UNITS = PPC * 2             # 6 units per core
FREE_IN = 5 * PW            # 2580
ALPHA1, ALPHA2 = 0.1, 1.5
SQ_ACT_DI = 3               # tap-rows 0..SQ_ACT_DI-1 squared on ACT, rest on DVE

_cache = {}


def _spatial64():
    co = np.arange(-PAD, PAD + 1, dtype=np.float64)
    gy, gx = np.meshgrid(co, co, indexing="ij")
    return np.exp(-(gx ** 2 + gy ** 2) / (2.0 * ALPHA2)).reshape(-1)


UROWS = 132 * PW            # dram elems per unit (132 padded rows)


def _build(a_out, a_tgt, repeat=1, sq_act_di=None, recip_mode=0, ablate=None, recip_fast=True):
    if sq_act_di is None:
        sq_act_di = SQ_ACT_DI
    import concourse.bass as bass
    import concourse.bacc as bacc
    import concourse.tile as tile
    from concourse import mybir

    f16, f32 = mybir.dt.float16, mybir.dt.float32
    AF = mybir.ActivationFunctionType
    gam = (a_out * a_out / (2.0 * ALPHA1), a_tgt * a_tgt / (2.0 * ALPHA1))

    nc = bacc.Bacc("TRN2", target_bir_lowering=False, debug=False,
                   num_devices=NCORES)
    x = nc.dram_tensor("x", [UNITS * UROWS + 8], f16, kind="ExternalInput").ap()
    idn = nc.dram_tensor("idn", [NTAP, 128, 128], f16,
                         kind="ExternalInput").ap()
    y = nc.dram_tensor("y", [128, PPC], f32, kind="ExternalOutput").ap()

    def win(t, off, dims):
        a = t[:]
        return bass.AP(a.tensor, a.offset + off, [list(a.ap[0])] + dims)

    def dram_win(unit, off):
        a = x[:]
        return bass.AP(a.tensor, a.offset + unit * UROWS + off,
                       [[PW, 128], [1, FREE_IN]])

    with tile.TileContext(nc) as tc, ExitStack() as ctx:
        cpool = ctx.enter_context(tc.tile_pool(name="const", bufs=1))
        inp = ctx.enter_context(tc.tile_pool(name="inp", bufs=4))
        work = ctx.enter_context(tc.tile_pool(name="work", bufs=8))
        acc = ctx.enter_context(tc.tile_pool(name="acc", bufs=4, space="PSUM"))
        fin = ctx.enter_context(tc.tile_pool(name="fin", bufs=3))

        ident = cpool.tile([128, NTAP * 128], f16)
        for k in range(NTAP):
            nc.gpsimd.dma_start(ident[:, k * 128:(k + 1) * 128], idn[k])
        loss_sb = cpool.tile([128, PPC], f32)

        pre = {}
        if ablate == "dma":
            for unit in range(UNITS):
                pxa = cpool.tile([128, FREE_IN], f16, tag=f"pxa{unit}")
                nc.gpsimd.dma_start(pxa[:], dram_win(unit, 0))
                pxb = cpool.tile([128, FREE_IN], f16, tag=f"pxb{unit}")
                nc.gpsimd.dma_start(pxb[:], dram_win(unit, 1))
                pre[unit] = (pxa, pxb)

        def body(_iv=None):
          if ablate == "empty":
              nc.vector.tensor_copy(loss_sb[:, 0:1], loss_sb[:, 1:2])
              return
          for pair in range(PPC if ablate != "p1" else 1):
            per_img = []
            for img in range(2):
                unit = pair * 2 + img
                if ablate == "dma":
                    xa, xb = pre[unit]
                else:
                    xa = inp.tile([128, FREE_IN], f16, tag="xa")
                    nc.gpsimd.dma_start(xa[:], dram_win(unit, 0))
                    xb = inp.tile([128, FREE_IN], f16, tag="xb")
                    nc.gpsimd.dma_start(xb[:], dram_win(unit, 1))

                Wp = acc.tile([128, W], f32, tag="W")
                Up = acc.tile([128, W], f32, tag="U")
                for di in range(KS):
                    uc = work.tile([128, KS * W], f16, tag="u")
                    for g in (0, 1):     # dj parity groups
                        njs = 3 if g == 0 else 2
                        src_t = xa if g == 0 else xb
                        src = win(src_t, di * PW + g * 0, [[2, njs], [1, W]])
                        cen = win(xa, 2 * PW + 2, [[0, njs], [1, W]])
                        outap = win(uc, g * W, [[2 * W, njs], [1, W]])
                        nc.vector.tensor_sub(outap, src, cen)
                    d2c = work.tile([128, KS * W], f16, tag="d2")
                    sl = slice(0, KS * W)
                    if di < sq_act_di:
                        nc.scalar.activation(d2c[:, sl], uc[:, sl], AF.Square)
                    else:
                        nc.vector.tensor_mul(d2c[:, sl], uc[:, sl], uc[:, sl])
                    if ablate != "exp":
                        nc.scalar.activation(d2c[:, sl], d2c[:, sl], AF.Exp,
                                             bias=0.0, scale=-gam[img])
                    if ablate != "m":
                        nc.vector.tensor_mul(uc[:, sl], d2c[:, sl], uc[:, sl])
                    for dj in range(KS):
                        k = di * KS + dj
                        if ablate == "mm" and k not in (0, NTAP - 1):
                            continue
                        ksl = slice(dj * W, (dj + 1) * W)
                        lt = ident[:, k * 128:(k + 1) * 128]
                        nc.tensor.matmul(Wp[:], lt, d2c[:, ksl],
                                         start=(k == 0), stop=(k == NTAP - 1))
                        nc.tensor.matmul(Up[:], lt, uc[:, ksl],
                                         start=(k == 0), stop=(k == NTAP - 1))

                if ablate == "fin":
                    per_img.append((None, xa, Wp, Up))
                elif recip_mode == 0:
                    rw = fin.tile([128, W], f32, tag="rw")
                    if recip_fast:
                        nc.vector.reciprocal_approx_fast(rw[:], Wp[:])
                    else:
                        nc.vector.reciprocal(rw[:], Wp[:])
                    t = fin.tile([128, W], f32, tag="t")
                    nc.vector.tensor_mul(t[:], Up[:], rw[:])
                    per_img.append((t, xa, None, None))
                else:
                    per_img.append((None, xa, Wp, Up))

            (t_o, xa_o, Wp_o, Up_o), (t_t, xa_t, Wp_t, Up_t) = per_img
            if ablate == "fin":
                dd = fin.tile([128, W], f32, tag="dd")
                nc.vector.tensor_sub(dd[:], Wp_o[:], Up_o[:])
                nc.vector.tensor_reduce(loss_sb[:, pair:pair + 1], dd[:],
                                        axis=mybir.AxisListType.X,
                                        op=mybir.AluOpType.add,
                                        apply_absolute_value=True)
                continue
            dc = fin.tile([128, W], f32, tag="dc")
            nc.vector.tensor_sub(dc, win(xa_o, 2 * PW + 2, [[1, W]]),
                                 win(xa_t, 2 * PW + 2, [[1, W]]))
            diff = fin.tile([128, W], f32, tag="diff")
            if recip_mode == 0:
                nc.vector.tensor_sub(diff[:], t_o[:], t_t[:])
                nc.vector.tensor_add(diff[:], diff[:], dc[:])
            else:
                P = fin.tile([128, W], f32, tag="P")
                nc.vector.tensor_mul(P[:], Wp_o[:], Wp_t[:])
                t1 = fin.tile([128, W], f32, tag="t1")
                nc.vector.tensor_mul(t1[:], Up_o[:], Wp_t[:])
                t2 = fin.tile([128, W], f32, tag="t2")
                nc.vector.tensor_mul(t2[:], Up_t[:], Wp_o[:])
                rw = fin.tile([128, W], f32, tag="rw")
                nc.vector.reciprocal(rw[:], P[:])
                nc.vector.tensor_sub(t1[:], t1[:], t2[:])
                nc.vector.tensor_mul(t1[:], t1[:], rw[:])
                nc.vector.tensor_add(diff[:], t1[:], dc[:])
            nc.vector.tensor_reduce(loss_sb[:, pair:pair + 1], diff[:],
                                    axis=mybir.AxisListType.X,
                                    op=mybir.AluOpType.add,
                                    apply_absolute_value=True)

        if repeat == 1:
            body()
        else:
            with tc.For_i(0, repeat, 1):
                body()
        nc.gpsimd.dma_start(y[:], loss_sb[:])

    nc.compile()
    return nc


def _host_shards(output, target):
    """-> (in_maps, idn) for run_bass_kernel_spmd."""
    s = _spatial64()
    idn = np.zeros((NTAP, 128, 128), np.float16)
    for k in range(NTAP):
        np.fill_diagonal(idn[k], np.float16(s[k]))

    xs = []
    for arr in (output, target):
        pad = np.pad(arr.reshape(NCH, H, W),
                     ((0, 0), (PAD, PAD), (PAD, PAD)), mode="reflect")
        xs.append(pad.astype(np.float16))

    in_maps = []
    for core in range(NCORES):
        xc = np.zeros(UNITS * UROWS + 8, np.float16)
        xv = xc[:UNITS * UROWS].reshape(UNITS, UROWS)
        for p in range(PPC):
            bc, rb = divmod(core * PPC + p, RB)
            for img in (0, 1):
                blk = xs[img][bc][rb * 128: rb * 128 + 132]
                xv[p * 2 + img] = blk.reshape(-1)
        in_maps.append({"x": xc, "idn": idn})
    return in_maps


def _numpy_fallback(output, target):
    def filt(img):
        a = 0.5 if img.min() < 0 else 1.0
        img01 = a * img + (0.5 if a == 0.5 else 0.0)
        pad = np.pad(img01, ((0, 0), (0, 0), (PAD, PAD), (PAD, PAD)),
                     mode="reflect")
        pat = np.stack([pad[:, :, i:i + H, j:j + W]
                        for i in range(KS) for j in range(KS)], 2)
        cen = img01[:, :, None]
        s = _spatial64()[None, None, :, None, None]
        w = np.exp(-(pat - cen) ** 2 / (2 * ALPHA1)) * s
        return (w * pat).sum(2) / (w.sum(2) + 1e-8)

    o = filt(output.astype(np.float64))
    t = filt(target.astype(np.float64))
    return np.float32(np.abs(o - t).mean())


def kernel(output, target):
    from concourse.bass_utils import run_bass_kernel_spmd

    output = np.asarray(output, np.float32)
    target = np.asarray(target, np.float32)
    a_o = 0.5 if output.min() < 0 else 1.0
    a_t = 0.5 if target.min() < 0 else 1.0
    if a_o != a_t:
        return _numpy_fallback(output, target)

    key = (a_o, a_t)
    if key not in _cache:
        _cache[key] = _build_sym(a_o, a_t, sq_act=False)
    nc = _cache[key]

    in_maps = _host_shards_sym(output, target)
    res = run_bass_kernel_spmd(nc, in_maps, list(range(NCORES)))
    total = np.float64(0.0)
    for r in res.results:
        total += r["y"].astype(np.float64).sum()
    loss = a_o * total / (B * C * H * W)
    return np.float32(loss)


# ---------------- symmetric (half-tap) implementation ----------------
EW = 514                      # extended x-domain per half-tap: x in [-2,512) or [0,514)
HALF = [(0, 1), (0, 2)] + [(di, dj) for di in (1, 2) for dj in (-2, -1, 0, 1, 2)]
NH = len(HALF)                # 12


def _slot(di, dj):
    if di == 0:
        return 0 if dj == 1 else 1
    return 2 + (di - 1) * 5 + (dj + 2)


def _x0(dj):                  # domain start: x0 = -2 if dj>0 else 0
    return -2 if dj > 0 else 0


def _sym_consts(center=1.0):
    """idn2 [35,128,128], sidn [20,2,128] f16 lhsT constants."""
    s64 = _spatial64().reshape(KS, KS)

    def sv(di, dj):
        return np.float16(s64[di + PAD, dj + PAD])

    idn2 = np.zeros((35, 128, 128), np.float16)
    np.fill_diagonal(idn2[0], np.float16(s64[PAD, PAD] * center))  # center
    for t, (di, dj) in enumerate(HALF):
        np.fill_diagonal(idn2[1 + t], sv(di, dj))               # diag(s_t)
        if di > 0:                                              # band B_di*s_t
            b = idn2[13 + (t - 2)]
            for m in range(di, 128):
                b[m - di, m] = sv(di, dj)
        # negated shifted set for U
        n = idn2[23 + t]
        if di == 0:
            np.fill_diagonal(n, -sv(di, dj))
        else:
            for m in range(di, 128):
                n[m - di, m] = -sv(di, dj)
    sidn = np.zeros((20, 12, 12), np.float16)
    for t, (di, dj) in enumerate(HALF):
        if di == 0:
            continue
        st = t - 2
        for u in range(UNITS):
            for j in range(2):          # strip row j -> unit-local out row j-2+di
                m = j - 2 + di
                if 0 <= m < 2:
                    sidn[st, 2 * u + j, 2 * u + m] = sv(di, dj)
                    sidn[10 + st, 2 * u + j, 2 * u + m] = -sv(di, dj)
    sel = np.zeros((12, UNITS * 128), np.float16)
    for u in range(UNITS):
        for j in range(2):
            sel[2 * u + j, u * 128 + j] = 1.0
    return idn2, sidn, sel


def _build_sym(a_out, a_tgt, repeat=1, sq_act=True, bw=2, bi=4, bs=1, bf=3):
    import concourse.bass as bass
    import concourse.bacc as bacc
    import concourse.tile as tile
    from concourse import mybir

    f16, f32 = mybir.dt.float16, mybir.dt.float32
    AF = mybir.ActivationFunctionType
    gam = (a_out * a_out / (2.0 * ALPHA1), a_tgt * a_tgt / (2.0 * ALPHA1))

    nc = bacc.Bacc("TRN2", target_bir_lowering=False, debug=False,
                   num_devices=NCORES)
    x = nc.dram_tensor("x", [UNITS * UROWS + 8], f16, kind="ExternalInput").ap()
    idn = nc.dram_tensor("idn", [35, 128, 128], f16, kind="ExternalInput").ap()
    sid = nc.dram_tensor("sid", [20, 12, 12], f16, kind="ExternalInput").ap()
    sel = nc.dram_tensor("sel", [12, UNITS * 128], f16,
                         kind="ExternalInput").ap()
    y = nc.dram_tensor("y", [128, PPC], f32, kind="ExternalOutput").ap()

    def win(t, off, dims):
        a = t[:]
        return bass.AP(a.tensor, a.offset + off, [list(a.ap[0])] + dims)

    def dram_ap(off, dims):
        a = x[:]
        return bass.AP(a.tensor, a.offset + off, dims)

    DJG = [(2, (0, 1, 2)), (1, (0, 1, 2)), (0, (1, 2)), (-1, (1, 2)),
           (-2, (1, 2))]

    with tile.TileContext(nc) as tc, ExitStack() as ctx:
        cpool = ctx.enter_context(tc.tile_pool(name="const", bufs=1))
        inp = ctx.enter_context(tc.tile_pool(name="inp", bufs=bi))
        work = ctx.enter_context(tc.tile_pool(name="work", bufs=bw))
        spool = ctx.enter_context(tc.tile_pool(name="spool", bufs=bs))
        acc = ctx.enter_context(tc.tile_pool(name="acc", bufs=3, space="PSUM"))
        accf = ctx.enter_context(tc.tile_pool(name="accf", bufs=1, space="PSUM"))
        fin = ctx.enter_context(tc.tile_pool(name="fin", bufs=bf))

        ident = cpool.tile([128, 35 * 128], f16)
        for k in range(35):
            nc.gpsimd.dma_start(ident[:, k * 128:(k + 1) * 128], idn[k])
        sids = cpool.tile([12, 20 * 12], f16)
        for k in range(20):
            nc.gpsimd.dma_start(sids[:, k * 12:(k + 1) * 12], sid[k])
        sels = cpool.tile([12, UNITS * 128], f16)
        nc.gpsimd.dma_start(sels[:], sel[:])
        ones = cpool.tile([128, W], f16)
        nc.vector.memset(ones[:], 1.0)
        loss_sb = cpool.tile([128, PPC], f32)

        def emit_u(dst, xa_t, xb_t, cen_t, cen_off, strip):
            for dj, dis in DJG:
                dis = tuple(di for di in dis if (not strip or di > 0))
                cnt, di0 = len(dis), ([di for di in dis if (not strip or di > 0)])[0]
                if strip:
                    t0 = (di0 - 1) * 5 + (dj + 2)
                else:
                    t0 = _slot(di0, dj)
                coff = 0 if dj > 0 else 2
                poff = dj if dj > 0 else (dj + 2)
                if poff % 2 == 0:
                    src_t, pbase = xa_t, poff
                else:
                    src_t, pbase = xb_t, poff - 1
                rbase = di0 if strip else (di0 + 2)
                src = win(src_t, rbase * PW + pbase, [[PW, cnt], [1, EW]])
                cen = win(cen_t, cen_off + coff, [[0, cnt], [1, EW]])
                out = win(dst, t0 * EW, [[5 * EW, cnt], [1, EW]])
                nc.vector.tensor_sub(out, src, cen)

        def body(_iv=None):
            # ---- strips first: inputs straight from DRAM, 2 rows per unit
            sin = spool.tile([12, 3 * PW], f16, tag="sin")
            sinb = spool.tile([12, 3 * PW], f16, tag="sinb")
            for uix in range(UNITS):
                nc.gpsimd.dma_start(
                    sin[2 * uix:2 * uix + 2, :],
                    dram_ap(uix * UROWS, [[PW, 2], [1, 3 * PW]]))
                nc.gpsimd.dma_start(
                    sinb[2 * uix:2 * uix + 2, :],
                    dram_ap(uix * UROWS + 1, [[PW, 2], [1, 3 * PW]]))
            su = spool.tile([12, 10 * EW], f16, tag="su")
            emit_u(su, sin, sinb, sin, 0, strip=True)
            sd2 = spool.tile([12, 10 * EW], f16, tag="sd2")
            nc.scalar.activation(sd2[:], su[:], AF.Derivative_Erf,
                                 bias=0.0, scale=float(np.sqrt(gam[0])))
            nc.vector.tensor_mul(su[:], sd2[:], su[:])

            fixW = accf.tile([12, W], f32, tag="fW")
            fixU = accf.tile([12, W], f32, tag="fU")
            stk = [t for t, (di, dj) in enumerate(HALF) if di > 0]
            for n, t in enumerate(stk):
                di, dj = HALF[t]
                st = t - 2
                o_s = st * EW + (2 - dj if dj > 0 else -dj)
                nc.tensor.matmul(fixW[:], sids[:, st * 12:(st + 1) * 12],
                                 sd2[:, o_s:o_s + W],
                                 start=(n == 0), stop=(n == len(stk) - 1))
                nc.tensor.matmul(fixU[:], sids[:, (10 + st) * 12:(11 + st) * 12],
                                 su[:, o_s:o_s + W],
                                 start=(n == 0), stop=(n == len(stk) - 1))
            fxw = spool.tile([12, W], f16, tag="fxw")
            nc.vector.tensor_copy(fxw[:], fixW[:])
            fxu = spool.tile([12, W], f16, tag="fxu")
            nc.vector.tensor_copy(fxu[:], fixU[:])

            for pair in range(PPC):
                per_img = []
                for img in range(2):
                    unit = pair * 2 + img
                    xa = inp.tile([128, FREE_IN], f16, tag="xa")
                    nc.gpsimd.dma_start(
                        xa[:], dram_ap(unit * UROWS, [[PW, 128], [1, FREE_IN]]))
                    xb = inp.tile([128, FREE_IN], f16, tag="xb")
                    nc.gpsimd.dma_start(
                        xb[:], dram_ap(unit * UROWS + 1,
                                       [[PW, 128], [1, FREE_IN]]))

                    u = work.tile([128, NH * EW], f16, tag="u")
                    emit_u(u, xa, xb, xa, 2 * PW, strip=False)
                    d2 = work.tile([128, NH * EW], f16, tag="d2")
                    nc.scalar.activation(d2[:], u[:], AF.Derivative_Erf,
                                         bias=0.0,
                                         scale=float(np.sqrt(gam[img])))
                    nc.vector.tensor_mul(u[:], d2[:], u[:])

                    Wp = acc.tile([128, W], f32, tag="W")
                    Up = acc.tile([128, W], f32, tag="U")
                    nc.tensor.matmul(Wp[:], ident[:, 0:128], ones[:],
                                     start=True, stop=False)
                    for t, (di, dj) in enumerate(HALF):
                        o_un = t * EW + (2 if dj > 0 else 0)
                        o_sh = t * EW + (2 - dj if dj > 0 else -dj)
                        lt_d = ident[:, (1 + t) * 128:(2 + t) * 128]
                        lt_sw = (lt_d if di == 0 else
                                 ident[:, (11 + t) * 128:(12 + t) * 128])
                        nc.tensor.matmul(Wp[:], lt_d, d2[:, o_un:o_un + W],
                                         start=False, stop=False)
                        nc.tensor.matmul(Wp[:], lt_sw, d2[:, o_sh:o_sh + W],
                                         start=False, stop=False)
                    for t, (di, dj) in enumerate(HALF):
                        o_un = t * EW + (2 if dj > 0 else 0)
                        o_sh = t * EW + (2 - dj if dj > 0 else -dj)
                        lt_d = ident[:, (1 + t) * 128:(2 + t) * 128]
                        lt_su = ident[:, (23 + t) * 128:(24 + t) * 128]
                        nc.tensor.matmul(Up[:], lt_d, u[:, o_un:o_un + W],
                                         start=(t == 0), stop=False)
                        nc.tensor.matmul(Up[:], lt_su, u[:, o_sh:o_sh + W],
                                         start=False, stop=False)
                    usel = sels[:, unit * 128:(unit + 1) * 128]
                    nc.tensor.matmul(Wp[:], usel, fxw[:],
                                     start=False, stop=True)
                    nc.tensor.matmul(Up[:], usel, fxu[:],
                                     start=False, stop=True)
                    per_img.append((xa, Wp, Up))

                (xa_o, Wp_o, Up_o), (xa_t2, Wp_t, Up_t) = per_img
                rw_o = fin.tile([128, W], f32, tag="rwo")
                nc.vector.reciprocal_approx_fast(rw_o[:], Wp_o[:])
                t_o = fin.tile([128, W], f32, tag="to")
                nc.vector.tensor_mul(t_o[:], Up_o[:], rw_o[:])
                rw_t = fin.tile([128, W], f32, tag="rwt")
                nc.vector.reciprocal_approx_fast(rw_t[:], Wp_t[:])
                t_t = fin.tile([128, W], f32, tag="tt")
                nc.vector.tensor_mul(t_t[:], Up_t[:], rw_t[:])
                dc = fin.tile([128, W], f32, tag="dc")
                nc.vector.tensor_sub(dc, win(xa_o, 2 * PW + 2, [[1, W]]),
                                     win(xa_t2, 2 * PW + 2, [[1, W]]))
                diff = fin.tile([128, W], f32, tag="diff")
                nc.vector.tensor_sub(diff[:], t_o[:], t_t[:])
                nc.vector.tensor_add(diff[:], diff[:], dc[:])
                nc.vector.tensor_reduce(loss_sb[:, pair:pair + 1], diff[:],
                                        axis=mybir.AxisListType.X,
                                        op=mybir.AluOpType.add,
                                        apply_absolute_value=True)

        if repeat == 1:
            body()
        else:
            with tc.For_i(0, repeat, 1):
                body()
        nc.gpsimd.dma_start(y[:], loss_sb[:])

    nc.compile()
    return nc


def _host_shards_sym(output, target):
    # DErf emits (2/sqrt(pi))*exp(-gam*u^2); every W/U tap term carries that
    # factor, so the center identity must carry it too (U/W is then invariant)
    in_maps = _host_shards(output, target)
    idn2, sidn, sel = _sym_consts(center=2.0 / np.sqrt(np.pi))
    for m in in_maps:
        m["idn"] = idn2
        m["sid"] = sidn
        m["sel"] = sel
    return in_maps


# ---------------- v3: fp8 DoubleRow implementation ----------------
# r~ = DErf(sqrt(gam)*u) = (2/sqrt(pi)) * exp(-gam*u^2), emitted as fp8e4m3
# straight from the ACT engine.  Per half-tap t the W accumulation needs
# diag(w_t) @ r~_t  (lane-aligned)  and  band_t(w_t) @ r~_t(x-dj) (mirror);
# both fuse into ONE fp8 DoubleRow matmul (2 k-tiles, rhs k-tile stride -dj).
# U likewise with [diag(+w), band(-w)] applied to m = r~*u (fp8).
# w_t = fp8(s_t * sqrt(pi)/2) so w_t * r~ = s_t * exp(-gam u^2).
SEGW = 516                  # per-tap segment width: x in [-2, 514)
NSEG = 12
# seg order grouped by dj so emit_u can batch: (dj, [di...])
V3_GROUPS = [(1, (0, 1, 2)), (2, (0, 1, 2)), (0, (1, 2)), (-1, (1, 2)),
             (-2, (1, 2))]
V3_TAPS = [(di, dj) for dj, dis in V3_GROUPS for di in dis]   # 12 half-taps


# DoubleRow k-tile pairs must have an EVEN rhs stride (odd strides hang the
# PE).  Diag reads all sit at seg*SEGW+2 so any two pair up (stride 516);
# band reads sit at seg*SEGW+2-dj so only same-dj taps pair up.  Taps 2
# (di=2,dj=1) and 5 (di=2,dj=2) are left over -> plain fp8 matmuls.
V3_DIAG_PAIRS = [(0, 1), (2, 3), (4, 5), (6, 7), (8, 9), (10, 11)]
V3_BAND_PAIRS = [(0, 1), (3, 4), (6, 7), (8, 9), (10, 11)]
V3_BAND_SINGLES = [2, 5]


def _v3_mm_specs():
    """[(lhs_block, nblk, rhs_off, rhs_stride, dr)] for one accumulation path
    (W); U uses lhs_block + 24.  lhs blocks are 128-col units."""
    specs = []
    blk = 0
    for t0, t1 in V3_DIAG_PAIRS:
        specs.append((blk, 2, t0 * SEGW + 2, (t1 - t0) * SEGW, True))
        blk += 2
    for t0, t1 in V3_BAND_PAIRS:
        dj = V3_TAPS[t0][1]
        specs.append((blk, 2, t0 * SEGW + 2 - dj, (t1 - t0) * SEGW, True))
        blk += 2
    for t in V3_BAND_SINGLES:
        dj = V3_TAPS[t][1]
        specs.append((blk, 1, t * SEGW + 2 - dj, 0, False))
        blk += 1
    assert blk == 24
    return specs


def _v3_consts():
    """wp [48,128,128] fp8-as-uint8 lhsT blocks, ident16 [128,128] f16,
    sidn [20,12,12] f16 (scaled by sqrt(pi)/2), sel [12, UNITS*128] f16."""
    import ml_dtypes
    s64 = _spatial64().reshape(KS, KS)
    lam_inv = np.sqrt(np.pi) / 2.0

    def w8of(t):
        di, dj = V3_TAPS[t]
        return ml_dtypes.float8_e4m3(s64[di + PAD, dj + PAD] * lam_inv)

    def diag(t, sign=1.0):
        m = np.zeros((128, 128), dtype=ml_dtypes.float8_e4m3)
        v = ml_dtypes.float8_e4m3(sign * float(w8of(t)))
        for q in range(128):
            m[q, q] = v
        return m

    def band(t, sign=1.0):
        m = np.zeros((128, 128), dtype=ml_dtypes.float8_e4m3)
        di = V3_TAPS[t][0]
        v = ml_dtypes.float8_e4m3(sign * float(w8of(t)))
        for q in range(di, 128):
            m[q - di, q] = v
        return m

    blocks = []
    for sign in (1.0, -1.0):        # W pass then U pass (U bands negated)
        for t0, t1 in V3_DIAG_PAIRS:
            blocks += [diag(t0), diag(t1)]
        for t0, t1 in V3_BAND_PAIRS:
            blocks += [band(t0, sign), band(t1, sign)]
        for t in V3_BAND_SINGLES:
            blocks.append(band(t, sign))
    wp = np.stack(blocks)           # [48,128,128]

    ident16 = np.zeros((128, 128), np.float16)
    np.fill_diagonal(ident16, np.float16(1.0))

    sidn = np.zeros((20, 12, 12), np.float16)
    for t, (di, dj) in enumerate(HALF):
        if di == 0:
            continue
        st = t - 2
        sv = np.float16(s64[di + PAD, dj + PAD] * lam_inv)
        for u in range(UNITS):
            for j in range(2):
                m = j - 2 + di
                if 0 <= m < 2:
                    sidn[st, 2 * u + j, 2 * u + m] = sv
                    sidn[10 + st, 2 * u + j, 2 * u + m] = -sv
    sel = np.zeros((12, UNITS * 128), np.float16)
    for u in range(UNITS):
        for j in range(2):
            sel[2 * u + j, u * 128 + j] = 1.0
    return wp.view(np.uint8), ident16, sidn, sel


def _build_v3(a_out, a_tgt, repeat=1, m_dve_segs=6, unroll=False, ablate=None):
    import concourse.bass as bass
    import concourse.bacc as bacc
    import concourse.tile as tile
    from concourse import mybir

    f16, f32 = mybir.dt.float16, mybir.dt.float32
    f8 = mybir.dt.float8e4
    AF = mybir.ActivationFunctionType
    PM = mybir.MatmulPerfMode.DoubleRow
    gam = (a_out * a_out / (2.0 * ALPHA1), a_tgt * a_tgt / (2.0 * ALPHA1))
    sc = (float(np.sqrt(gam[0])), float(np.sqrt(gam[1])))

    nc = bacc.Bacc("TRN2", target_bir_lowering=False, debug=False,
                   num_devices=NCORES)
    x = nc.dram_tensor("x", [UNITS * UROWS + 8], f16, kind="ExternalInput").ap()
    wpd = nc.dram_tensor("wp", [48, 128, 128], f8, kind="ExternalInput").ap()
    idn = nc.dram_tensor("idn", [128, 128], f16, kind="ExternalInput").ap()
    sid = nc.dram_tensor("sid", [20, 12, 12], f16, kind="ExternalInput").ap()
    sel = nc.dram_tensor("sel", [12, UNITS * 128], f16,
                         kind="ExternalInput").ap()
    y = nc.dram_tensor("y", [128, PPC], f32, kind="ExternalOutput").ap()

    def win(t, off, dims):
        a = t[:]
        return bass.AP(a.tensor, a.offset + off, [list(a.ap[0])] + dims)

    def dram_ap(off, dims):
        a = x[:]
        return bass.AP(a.tensor, a.offset + off, dims)

    with tile.TileContext(nc) as tc, ExitStack() as ctx:
        cpool = ctx.enter_context(tc.tile_pool(name="const", bufs=1))
        inp = ctx.enter_context(tc.tile_pool(name="inp", bufs=4))
        work = ctx.enter_context(tc.tile_pool(name="work", bufs=2))
        spool = ctx.enter_context(tc.tile_pool(name="spool", bufs=1))
        acc = ctx.enter_context(tc.tile_pool(name="acc", bufs=2, space="PSUM"))
        accf = ctx.enter_context(tc.tile_pool(name="accf", bufs=1, space="PSUM"))
        fin = ctx.enter_context(tc.tile_pool(name="fin", bufs=3))

        wps = cpool.tile([128, 48 * 128], f8)
        for k in range(48):
            nc.gpsimd.dma_start(wps[:, k * 128:(k + 1) * 128], wpd[k])
        mm_specs = _v3_mm_specs()
        id16 = cpool.tile([128, 128], f16)
        nc.gpsimd.dma_start(id16[:], idn[:])
        sids = cpool.tile([12, 20 * 12], f16)
        for k in range(20):
            nc.gpsimd.dma_start(sids[:, k * 12:(k + 1) * 12], sid[k])
        sels = cpool.tile([12, UNITS * 128], f16)
        nc.gpsimd.dma_start(sels[:], sel[:])
        ones = cpool.tile([128, W], f16)
        nc.vector.memset(ones[:], 1.0)
        loss_sb = cpool.tile([128, PPC], f32)

        def emit_u_v3(dst, xa_t, xb_t):
            # write u for 12 segs; position k in a segment holds x = k-2.
            # src col (padded) = k+dj -> base (di+2)*PW+dj; cen col = k.
            # dj>0 groups shorten so reads stay inside the xa row window.
            seg = 0
            for dj, dis in V3_GROUPS:
                cnt, di0 = len(dis), dis[0]
                ln = SEGW - max(dj, 0)
                base = (di0 + 2) * PW + dj
                if dj % 2 == 0:
                    src_t, b = xa_t, base
                else:
                    src_t, b = xb_t, base - 1
                src = win(src_t, b, [[PW, cnt], [1, ln]])
                cen = win(xa_t, 2 * PW, [[0, cnt], [1, ln]])
                out = win(dst, seg * SEGW, [[SEGW, cnt], [1, ln]])
                nc.vector.tensor_sub(out, src, cen)
                seg += cnt

        ab = set((ablate or "").split(","))
        skip_strips = "nostrip" in ab
        skip_mm = "nomm" in ab
        skip_m8 = "nom8" in ab
        skip_fin = "nofin" in ab
        skip_act = "noact" in ab
        skip_emit = "noemit" in ab
        skip_dma = "nodma" in ab

        pre = {}
        if skip_dma:
            for unit in range(UNITS):
                pxa = cpool.tile([128, FREE_IN], f16, tag=f"pxa{unit}")
                nc.gpsimd.dma_start(pxa[:], dram_ap(unit * UROWS,
                                                    [[PW, 128], [1, FREE_IN]]))
                pxb = cpool.tile([128, FREE_IN], f16, tag=f"pxb{unit}")
                nc.gpsimd.dma_start(pxb[:], dram_ap(unit * UROWS + 1,
                                                    [[PW, 128], [1, FREE_IN]]))
                pre[unit] = (pxa, pxb)

        def strips_body():
            # ---- strips (fp16 path, 12 partitions, all 6 units at once)
            sin = spool.tile([12, 3 * PW], f16, tag="sin")
            sinb = spool.tile([12, 3 * PW], f16, tag="sinb")
            for uix in range(UNITS):
                nc.gpsimd.dma_start(
                    sin[2 * uix:2 * uix + 2, :],
                    dram_ap(uix * UROWS, [[PW, 2], [1, 3 * PW]]))
                nc.gpsimd.dma_start(
                    sinb[2 * uix:2 * uix + 2, :],
                    dram_ap(uix * UROWS + 1, [[PW, 2], [1, 3 * PW]]))
            su = spool.tile([12, 10 * EW], f16, tag="su")
            # strip emit (EW=514 layout, di>0 taps only) - reuse old helper
            for dj, dis in [(2, (1, 2)), (1, (1, 2)), (0, (1, 2)),
                            (-1, (1, 2)), (-2, (1, 2))]:
                cnt, di0 = len(dis), dis[0]
                t0 = (di0 - 1) * 5 + (dj + 2)
                coff = 0 if dj > 0 else 2
                poff = dj if dj > 0 else (dj + 2)
                if poff % 2 == 0:
                    src_t, pbase = sin, poff
                else:
                    src_t, pbase = sinb, poff - 1
                src = win(src_t, di0 * PW + pbase, [[PW, cnt], [1, EW]])
                cen = win(sin, coff, [[0, cnt], [1, EW]])
                out = win(su, t0 * EW, [[5 * EW, cnt], [1, EW]])
                nc.vector.tensor_sub(out, src, cen)
            srs = spool.tile([12, 10 * EW], f16, tag="srs")
            nc.scalar.activation(srs[:], su[:], AF.Derivative_Erf,
                                 bias=0.0, scale=sc[0])
            nc.vector.tensor_mul(su[:], srs[:], su[:])

            fixW = accf.tile([12, W], f32, tag="fW")
            fixU = accf.tile([12, W], f32, tag="fU")
            stk = [t for t, (di, dj) in enumerate(HALF) if di > 0]
            for n, t in enumerate(stk):
                di, dj = HALF[t]
                st = t - 2
                o_s = st * EW + (2 - dj if dj > 0 else -dj)
                nc.tensor.matmul(fixW[:], sids[:, st * 12:(st + 1) * 12],
                                 srs[:, o_s:o_s + W],
                                 start=(n == 0), stop=(n == len(stk) - 1))
                nc.tensor.matmul(fixU[:], sids[:, (10 + st) * 12:(11 + st) * 12],
                                 su[:, o_s:o_s + W],
                                 start=(n == 0), stop=(n == len(stk) - 1))
            fxw = spool.tile([12, W], f16, tag="fxw")
            nc.scalar.activation(fxw[:], fixW[:], AF.Copy)
            fxu = spool.tile([12, W], f16, tag="fxu")
            nc.scalar.activation(fxu[:], fixU[:], AF.Copy)
            return fxw, fxu

        def body(_iv=None):
            if not skip_strips:
                fxw, fxu = strips_body()

            for pair in range(PPC):
                per_img = []
                for img in range(2):
                    unit = pair * 2 + img
                    if skip_dma:
                        xa, xb = pre[unit]
                    else:
                        xa = inp.tile([128, FREE_IN], f16, tag="xa")
                        nc.gpsimd.dma_start(
                            xa[:], dram_ap(unit * UROWS,
                                           [[PW, 128], [1, FREE_IN]]))
                        xb = inp.tile([128, FREE_IN], f16, tag="xb")
                        nc.gpsimd.dma_start(
                            xb[:], dram_ap(unit * UROWS + 1,
                                           [[PW, 128], [1, FREE_IN]]))

                    u = work.tile([128, NSEG * SEGW], f16, tag="u")
                    if not skip_emit:
                        emit_u_v3(u, xa, xb)
                    r8 = work.tile([128, NSEG * SEGW], f8, tag="r8")
                    if not skip_act:
                        nc.scalar.activation(r8[:], u[:], AF.Derivative_Erf,
                                             bias=0.0, scale=sc[img])
                    m8 = work.tile([128, NSEG * SEGW], f8, tag="m8")
                    if not skip_m8:
                        kd = m_dve_segs * SEGW
                        if kd > 0:
                            nc.vector.tensor_mul(m8[:, 0:kd], r8[:, 0:kd],
                                                 u[:, 0:kd])
                        if kd < NSEG * SEGW:
                            nc.gpsimd.tensor_mul(m8[:, kd:NSEG * SEGW],
                                                 r8[:, kd:NSEG * SEGW],
                                                 u[:, kd:NSEG * SEGW])
                    else:
                        m8 = r8

                    Wp = acc.tile([128, W], f32, tag="W")
                    Up = acc.tile([128, W], f32, tag="U")
                    nc.tensor.matmul(Wp[:], id16[:], ones[:],
                                     start=True, stop=skip_mm and skip_strips)
                    if not skip_mm:
                        for psum, src, lhs_base, st0 in ((Wp, r8, 0, False),
                                                         (Up, m8, 24, True)):
                            for n, (blk, nb, roff, rstr, dr) in \
                                    enumerate(mm_specs):
                                if dr:
                                    lhs = win(wps, (lhs_base + blk) * 128,
                                              [[128, 2], [1, 128]])
                                    rhs = win(src, roff, [[rstr, 2], [1, W]])
                                    nc.tensor.matmul(psum[:], lhs, rhs,
                                                     start=(st0 and n == 0),
                                                     stop=False, perf_mode=PM)
                                else:
                                    lhs = win(wps, (lhs_base + blk) * 128,
                                              [[1, 128]])
                                    rhs = win(src, roff, [[1, W]])
                                    nc.tensor.matmul(psum[:], lhs, rhs,
                                                     start=False, stop=False)
                    elif not skip_strips:
                        nc.tensor.matmul(Up[:], id16[:], ones[:],
                                         start=True, stop=False)
                    if not skip_strips:
                        usel = sels[:, unit * 128:(unit + 1) * 128]
                        nc.tensor.matmul(Wp[:], usel, fxw[:],
                                         start=False, stop=True)
                        nc.tensor.matmul(Up[:], usel, fxu[:],
                                         start=False, stop=True)
                    elif not skip_mm:
                        nc.tensor.matmul(Wp[:], id16[:], ones[:],
                                         start=False, stop=True)
                        nc.tensor.matmul(Up[:], id16[:], ones[:],
                                         start=False, stop=True)
                    per_img.append((xa, Wp, Up))

                if skip_fin:
                    nc.vector.memset(loss_sb[:, pair:pair + 1], 0.0)
                    continue
                (xa_o, Wp_o, Up_o), (xa_t2, Wp_t, Up_t) = per_img
                rw_o = fin.tile([128, W], f32, tag="rwo")
                nc.vector.reciprocal_approx_fast(rw_o[:], Wp_o[:])
                t_o = fin.tile([128, W], f16, tag="to")
                nc.vector.tensor_mul(t_o[:], Up_o[:], rw_o[:])
                rw_t = fin.tile([128, W], f32, tag="rwt")
                nc.vector.reciprocal_approx_fast(rw_t[:], Wp_t[:])
                t_t = fin.tile([128, W], f16, tag="tt")
                nc.vector.tensor_mul(t_t[:], Up_t[:], rw_t[:])
                dc = fin.tile([128, W], f16, tag="dc")
                nc.gpsimd.tensor_sub(dc, win(xa_o, 2 * PW + 2, [[1, W]]),
                                     win(xa_t2, 2 * PW + 2, [[1, W]]))
                A = fin.tile([128, W], f16, tag="A")
                nc.gpsimd.tensor_add(A[:], dc[:], t_o[:])
                diff = fin.tile([128, W], f16, tag="diff")
                nc.gpsimd.tensor_sub(diff[:], A[:], t_t[:])
                junk = fin.tile([128, W], f16, tag="junk")
                nc.scalar.activation(junk[:], diff[:], AF.Abs,
                                     accum_out=loss_sb[:, pair:pair + 1])

        if repeat == 1:
            body()
        elif unroll:
            for _ in range(repeat):
                body()
        else:
            with tc.For_i(0, repeat, 1):
                body()
        nc.gpsimd.dma_start(y[:], loss_sb[:])

    nc.compile()
    return nc


def _host_shards_v3(output, target):
    in_maps = _host_shards(output, target)
    wp8, ident16, sidn, sel = _v3_consts()
    for m in in_maps:
        del m["idn"]
        m["wp"] = wp8
        m["idn"] = ident16
        m["sid"] = sidn
        m["sel"] = sel
    return in_maps


# ---------------- v4: fp16 m + fp8-DR W + SP-hwdge DMA + packed strips ------
# Division of labour per unit:
#   emit u (DVE f16 2x)  ->  r16 = DErf (ACT f16)  ->  m16 = r16*u in-place
#   (DVE f16 2x)  ;  r8 = SWDGE cast-DMA of r16 (fp8)
#   W  psum: ones + 11 fp8 DoubleRow + 2 plain fp8 (mm_specs)   [cheap]
#   U  psum: 24 fp16 matmuls (diag +w, band -w), w == float(fp8(s*sqrt(pi)/2))
#            so W and U use bit-identical per-tap weights.
#   strips: all 6 units x 2 halo rows x 10 taps packed into 120 partitions,
#   free dim 516 -> DErf costs 0.6us instead of 4.5us; per-unit fix matmuls
#   replace the sidn/sel chain.
# All input DMAs issue from the SP engine (HWDGE) so the Pool engine never
# blocks the pipeline; only the r8 cast uses the (otherwise idle) SWDGE ring.
V3_STRIP_TAPS = [(di, dj) for (di, dj) in V3_TAPS if di > 0]   # 10 taps


def _v4_consts():
    import ml_dtypes
    s64 = _spatial64().reshape(KS, KS)
    lam_inv = np.sqrt(np.pi) / 2.0

    def w8val(t):
        di, dj = V3_TAPS[t]
        return float(ml_dtypes.float8_e4m3(s64[di + PAD, dj + PAD] * lam_inv))

    wp8, ident16, _sid, _sel = _v3_consts()         # 48 blocks: W 0-23, U 24-47

    wu16 = np.zeros((24, 128, 128), np.float16)
    for t in range(NSEG):
        di = V3_TAPS[t][0]
        v = np.float16(w8val(t))
        np.fill_diagonal(wu16[t], v)
        for q in range(di, 128):
            wu16[12 + t, q - di, q] = -v

    # strip fix lhsT: [120, UNITS*128] for W and U
    fxw = np.zeros((120, UNITS * 128), np.float16)
    fxu = np.zeros((120, UNITS * 128), np.float16)
    for ti, (di, dj) in enumerate(V3_STRIP_TAPS):
        t = V3_TAPS.index((di, dj))
        v = np.float16(w8val(t))
        for u in range(UNITS):
            for jj in range(2):
                m = jj - 2 + di
                if 0 <= m < 2:
                    p = ti * 12 + 2 * u + jj
                    fxw[p, u * 128 + m] = v
                    fxu[p, u * 128 + m] = -v
    return wp8, ident16, wu16, fxw, fxu


def _build_v4(a_out, a_tgt, repeat=1, unroll=False, ablate=None):
    import concourse.bass as bass
    import concourse.bacc as bacc
    import concourse.tile as tile
    from concourse import mybir

    f16, f32 = mybir.dt.float16, mybir.dt.float32
    f8 = mybir.dt.float8e4
    AF = mybir.ActivationFunctionType
    PM = mybir.MatmulPerfMode.DoubleRow
    gam = (a_out * a_out / (2.0 * ALPHA1), a_tgt * a_tgt / (2.0 * ALPHA1))
    sc = (float(np.sqrt(gam[0])), float(np.sqrt(gam[1])))

    nc = bacc.Bacc("TRN2", target_bir_lowering=False, debug=False,
                   num_devices=NCORES)
    x = nc.dram_tensor("x", [UNITS * UROWS + 8], f16, kind="ExternalInput").ap()
    wpd = nc.dram_tensor("wp", [48, 128, 128], f8, kind="ExternalInput").ap()
    wud = nc.dram_tensor("wu", [24, 128, 128], f16, kind="ExternalInput").ap()
    idn = nc.dram_tensor("idn", [128, 128], f16, kind="ExternalInput").ap()
    fxwd = nc.dram_tensor("fxw", [120, UNITS * 128], f16,
                          kind="ExternalInput").ap()
    fxud = nc.dram_tensor("fxu", [120, UNITS * 128], f16,
                          kind="ExternalInput").ap()
    y = nc.dram_tensor("y", [128, PPC], f32, kind="ExternalOutput").ap()

    def win(t, off, dims):
        a = t[:]
        return bass.AP(a.tensor, a.offset + off, [list(a.ap[0])] + dims)

    def dram_ap(off, dims):
        a = x[:]
        return bass.AP(a.tensor, a.offset + off, dims)

    with tile.TileContext(nc) as tc, ExitStack() as ctx:
        cpool = ctx.enter_context(tc.tile_pool(name="const", bufs=1))
        inp = ctx.enter_context(tc.tile_pool(name="inp", bufs=4))
        work = ctx.enter_context(tc.tile_pool(name="work", bufs=2))
        spool = ctx.enter_context(tc.tile_pool(name="spool", bufs=2))
        acc = ctx.enter_context(tc.tile_pool(name="acc", bufs=2, space="PSUM"))
        fin = ctx.enter_context(tc.tile_pool(name="fin", bufs=3))

        wps = cpool.tile([128, 48 * 128], f8)
        for k in range(48):
            nc.gpsimd.dma_start(wps[:, k * 128:(k + 1) * 128], wpd[k])
        id16 = cpool.tile([128, 128], f16)
        nc.gpsimd.dma_start(id16[:], idn[:])
        fxws = cpool.tile([120, UNITS * 128], f16)
        nc.gpsimd.dma_start(fxws[:], fxwd[:])
        fxus = cpool.tile([120, UNITS * 128], f16)
        nc.gpsimd.dma_start(fxus[:], fxud[:])
        ones = cpool.tile([128, W], f16)
        nc.vector.memset(ones[:], 1.0)
        loss_sb = cpool.tile([128, PPC], f32)
        mm_specs = _v3_mm_specs()

        ab = set((ablate or "").split(","))
        skip_strips = "nostrip" in ab
        skip_mm = "nomm" in ab
        skip_m = "nom8" in ab
        skip_fin = "nofin" in ab
        skip_cast = "nocast" in ab

        def emit_u_v3(dst, xa_t, xb_t):
            seg = 0
            for dj, dis in V3_GROUPS:
                cnt, di0 = len(dis), dis[0]
                ln = SEGW - max(dj, 0)
                base = (di0 + 2) * PW + dj
                if dj % 2 == 0:
                    src_t, b = xa_t, base
                else:
                    src_t, b = xb_t, base - 1
                src = win(src_t, b, [[PW, cnt], [1, ln]])
                cen = win(xa_t, 2 * PW, [[0, cnt], [1, ln]])
                out = win(dst, seg * SEGW, [[SEGW, cnt], [1, ln]])
                nc.vector.tensor_sub(out, src, cen)
                seg += cnt

        def strips_body():
            # Tap-tailored loads: row (ti,u,jj), position k (x=k-2) holds
            #   S1 = I(block row jj+di, col k)   S2 = I(block row jj, col k-dj)
            # so su2 = S1-S2 = u_t(y'=jj-2, x-dj) in ONE [120]-partition op.
            s1t = spool.tile([120, SEGW], f16, tag="s1t")
            s2t = spool.tile([120, SEGW], f16, tag="s2t")
            for ti, (di, dj) in enumerate(V3_STRIP_TAPS):
                nc.sync.dma_start(
                    s1t[ti * 12:(ti + 1) * 12, 2:SEGW],
                    dram_ap(di * PW + 2,
                            [[UROWS, UNITS], [PW, 2], [1, SEGW - 2]]))
                nc.sync.dma_start(
                    s2t[ti * 12:(ti + 1) * 12, 2:SEGW],
                    dram_ap(2 - dj,
                            [[UROWS, UNITS], [PW, 2], [1, SEGW - 2]]))
            su2 = spool.tile([120, SEGW], f16, tag="su2")
            nc.vector.tensor_sub(su2[:, 2:SEGW], s1t[:, 2:SEGW],
                                 s2t[:, 2:SEGW])
            srs2 = spool.tile([120, SEGW], f16, tag="srs2")
            nc.scalar.activation(srs2[:, 2:SEGW], su2[:, 2:SEGW],
                                 AF.Derivative_Erf, bias=0.0, scale=sc[0])
            nc.vector.tensor_mul(su2[:, 2:SEGW], srs2[:, 2:SEGW],
                                 su2[:, 2:SEGW])
            return su2, srs2

        def body(_iv=None):
            if not skip_strips:
                su2, srs2 = strips_body()

            for pair in range(PPC):
                per_img = []
                for img in range(2):
                    unit = pair * 2 + img
                    xa = inp.tile([128, FREE_IN], f16, tag="xa")
                    nc.sync.dma_start(
                        xa[:], dram_ap(unit * UROWS,
                                       [[PW, 128], [1, FREE_IN]]))
                    xb = inp.tile([128, FREE_IN], f16, tag="xb")
                    nc.sync.dma_start(
                        xb[:], dram_ap(unit * UROWS + 1,
                                       [[PW, 128], [1, FREE_IN]]))

                    u = work.tile([128, NSEG * SEGW], f16, tag="u")
                    emit_u_v3(u, xa, xb)
                    r16 = work.tile([128, NSEG * SEGW], f16, tag="r16")
                    nc.scalar.activation(r16[:], u[:], AF.Derivative_Erf,
                                         bias=0.0, scale=sc[img])
                    r8 = work.tile([128, NSEG * SEGW], f8, tag="r8")
                    if not skip_cast:
                        nc.gpsimd.dma_start(r8[:], r16[:])
                    m8 = work.tile([128, NSEG * SEGW], f8, tag="m8")
                    if not skip_m:
                        nc.vector.tensor_mul(u[:], r16[:], u[:])  # m16 in u
                        nc.gpsimd.dma_start(m8[:], u[:])          # cast to fp8

                    Wp = acc.tile([128, W], f32, tag="W")
                    Up = acc.tile([128, W], f32, tag="U")
                    nc.tensor.matmul(Wp[:], id16[:], ones[:],
                                     start=True, stop=False)
                    if skip_mm:
                        nc.tensor.matmul(Up[:], id16[:], ones[:],
                                         start=True, stop=False)
                    else:
                        for psum, src, lb, st0 in ((Wp, r8, 0, False),
                                                   (Up, m8, 24, True)):
                            for n, (blk, nb, roff, rstr, dr) in \
                                    enumerate(mm_specs):
                                if dr:
                                    lhs = win(wps, (lb + blk) * 128,
                                              [[128, 2], [1, 128]])
                                    rhs = win(src, roff, [[rstr, 2], [1, W]])
                                    nc.tensor.matmul(psum[:], lhs, rhs,
                                                     start=(st0 and n == 0),
                                                     stop=False, perf_mode=PM)
                                else:
                                    lhs = win(wps, (lb + blk) * 128,
                                              [[1, 128]])
                                    rhs = win(src, roff, [[1, W]])
                                    nc.tensor.matmul(psum[:], lhs, rhs,
                                                     start=False, stop=False)
                    if not skip_strips:
                        fw = bass.AP(fxws[:].tensor,
                                     fxws[:].offset + unit * 128,
                                     [list(fxws[:].ap[0]), [1, 128]])
                        fu = bass.AP(fxus[:].tensor,
                                     fxus[:].offset + unit * 128,
                                     [list(fxus[:].ap[0]), [1, 128]])
                        nc.tensor.matmul(Wp[:], fw, win(srs2, 2, [[1, W]]),
                                         start=False, stop=True)
                        nc.tensor.matmul(Up[:], fu, win(su2, 2, [[1, W]]),
                                         start=False, stop=True)
                    else:
                        nc.tensor.matmul(Wp[:], id16[:], ones[:],
                                         start=False, stop=True)
                        nc.tensor.matmul(Up[:], id16[:], ones[:],
                                         start=False, stop=True)
                    per_img.append((xa, Wp, Up))

                if skip_fin:
                    nc.vector.memset(loss_sb[:, pair:pair + 1], 0.0)
                    continue
                (xa_o, Wp_o, Up_o), (xa_t2, Wp_t, Up_t) = per_img
                rw_o = fin.tile([128, W], f32, tag="rwo")
                nc.vector.reciprocal_approx_fast(rw_o[:], Wp_o[:])
                t_o = fin.tile([128, W], f16, tag="to")
                nc.vector.tensor_mul(t_o[:], Up_o[:], rw_o[:])
                rw_t = fin.tile([128, W], f32, tag="rwt")
                nc.vector.reciprocal_approx_fast(rw_t[:], Wp_t[:])
                t_t = fin.tile([128, W], f16, tag="tt")
                nc.vector.tensor_mul(t_t[:], Up_t[:], rw_t[:])
                dc = fin.tile([128, W], f16, tag="dc")
                nc.vector.tensor_sub(dc, win(xa_o, 2 * PW + 2, [[1, W]]),
                                     win(xa_t2, 2 * PW + 2, [[1, W]]))
                A = fin.tile([128, W], f16, tag="A")
                nc.vector.tensor_add(A[:], dc[:], t_o[:])
                diff = fin.tile([128, W], f16, tag="diff")
                nc.vector.tensor_sub(diff[:], A[:], t_t[:])
                junk = fin.tile([128, W], f16, tag="junk")
                nc.scalar.activation(junk[:], diff[:], AF.Abs,
                                     accum_out=loss_sb[:, pair:pair + 1])

        if repeat == 1:
            body()
        elif unroll:
            for _ in range(repeat):
                body()
        else:
            with tc.For_i(0, repeat, 1):
                body()
        nc.gpsimd.dma_start(y[:], loss_sb[:])

    nc.compile()
    return nc


def _host_shards_v4(output, target):
    in_maps = _host_shards(output, target)
    wp8, ident16, wu16, fxw, fxu = _v4_consts()
    for m in in_maps:
        del m["idn"]
        m["wp"] = wp8
        m["wu"] = wu16
        m["idn"] = ident16
        m["fxw"] = fxw
        m["fxu"] = fxu
    return in_maps

